# revision 21
# baseline (speedup 1.0000x reference)
"""Trainium2 Bass kernel for nn_BLP_52467320487972 (retrieval_knn, L1 scores).

score[b, e] = -sum_d |query_sum[b, d] - E_embed[e, d]|,  E_embed = [other_emb[0]; ent_pkl @ proj_W.T]

Strategy (8 NeuronCores, entity-sharded, 5000(+pad) entities/core):
  host:   exact query_sum [32, 256] (tiny gather + normalize); score column 0.
  device: P.T = W.T @ A computed on the PE as two [128d, gsz] halves per
          entity group.  Uses sum|x| = 2*sum(relu(x)) - sum(x); relu(P.T - q_b)
          tensors are produced per query on three engines in parallel and
          column-reduced back through the PE:
    * DVE lane:   one fused tensor_scalar (op0=subtract, op1=max) per half
      at the 4x perf mode -> bf16 relu halves; optionally folded
      (tensor_tensor add, 2x) so the PE reduce is a single pass.
    * ACT lane:   activation(Relu, bias=-q) writes fp8e4 relu halves directly
      into a [128, 2, gsz] tile; a DoubleRow stair matmul reduces BOTH
      halves in one half-rate pass (contract 256).
    * Pool lane:  gpsimd tensor_scalar produces the second fp8 half for
      mixed queries.
    A one-column-of-2.0 "staircase" lhsT routes each column-sum into PSUM
    row b; scores accumulate in PSUM f32 and are copied out via SBUF.
  host:   the "- sum(x)" term == colsum_P[e] - qsum[b] is applied on host
          (colsum_P = A_bf16 @ rowsum(W_bf16), exact in f32);
          concat score columns from the 8 cores, negate, prepend column 0.
"""

import sys

for _p in ("/opt/trn_rl_repo", "/root/.axon_site/_ro/trn_rl_repo"):
    if _p not in sys.path:
        sys.path.append(_p)

import numpy as np
import ml_dtypes

NUM_ENT = 40000
NUM_REL = 100
EMBED_DIM = 256
FEAT_DIM = 768
BATCH = 32
N_CORES = 8
SHARD = NUM_ENT // N_CORES          # 5000
SHARD_PAD = 5120
GSIZES = [512, 1024, 1024, 1024, 1024, 392]   # sums to 5000
GOFFS = [sum(GSIZES[:i]) for i in range(len(GSIZES))]
N_GROUPS = len(GSIZES)
K_CHUNKS = FEAT_DIM // 128          # 6
EPS = 1e-12

# ---- query routing (sums to 32) ----
N_ACT8 = 2      # ACT produces both fp8 halves; PE DoubleRow 1-pass reduce
N_MIX8 = 12     # ACT h0 + Pool h1 (fp8); PE DoubleRow 1-pass reduce
N_DVE_F = 10    # DVE bf16 halves + DVE fold; PE bf16 1-pass reduce
N_DVE_U = BATCH - N_ACT8 - N_MIX8 - N_DVE_F   # DVE halves; PE bf16 2-pass

# P-copy (PSUM f32 -> SBUF bf16) engine per (group, half): 10 slots
#   'v' = DVE, 'a' = ACT  (GPSIMD cannot read PSUM)
COPY_ENGINES = ['v', 'a'] * 5
# score copy (PSUM f32 -> SBUF f32) engine per group: 5 slots
SCOPY_ENGINES = ['a'] * 5

PROJ_FP8 = False                    # bf16 projection by default

BF16 = ml_dtypes.bfloat16
FP8 = ml_dtypes.float8_e4m3

_CACHE = {}


def _build_program():
    import concourse.bacc as bacc
    import concourse.mybir as mybir
    import concourse.tile as tile

    f32 = mybir.dt.float32
    bf16 = mybir.dt.bfloat16
    fp8 = mybir.dt.float8e4
    AL = mybir.AluOpType
    AF = mybir.ActivationFunctionType
    DR = mybir.MatmulPerfMode.DoubleRow

    nc = bacc.Bacc("TRN2", target_bir_lowering=False, debug=False, num_devices=N_CORES)

    # a_t[p, k, e] = A[e, 128*k + p]
    a_t = nc.declare_dram_parameter("a_t", [128, K_CHUNKS, SHARD], bf16, isOutput=False)
    w_t = nc.declare_dram_parameter("w_t", [128, K_CHUNKS, EMBED_DIM], bf16, isOutput=False)
    qt = nc.declare_dram_parameter("qt", [128, 2, BATCH], f32, isOutput=False)
    qtn = nc.declare_dram_parameter("qtn", [128, 2, BATCH], f32, isOutput=False)
    st_out = nc.declare_dram_parameter("st_out", [BATCH, SHARD_PAD], f32, isOutput=True)

    R_ACT8 = list(range(N_ACT8))
    R_MIX8 = list(range(N_ACT8, N_ACT8 + N_MIX8))
    R_DVE_F = list(range(N_ACT8 + N_MIX8, N_ACT8 + N_MIX8 + N_DVE_F))
    R_DVE_U = list(range(N_ACT8 + N_MIX8 + N_DVE_F, BATCH))
    R_FP8 = R_ACT8 + R_MIX8

    with tile.TileContext(nc) as tc:
        with (
            tc.tile_pool(name="const", bufs=1) as const_pool,
            tc.tile_pool(name="p", bufs=4) as p_pool,
            tc.tile_pool(name="absd", bufs=16) as absd_pool,
            tc.tile_pool(name="abs8", bufs=18) as abs8_pool,
            tc.tile_pool(name="srp", bufs=2) as sr_pool,
            tc.tile_pool(name="trk", bufs=2) as trk_pool,
            tc.tile_pool(name="psumt", bufs=2, space="PSUM") as psumt_pool,
            tc.tile_pool(name="psums", bufs=2, space="PSUM") as psums_pool,
        ):
            # ---- PE p-state warmup: tiny matmuls with no DMA deps ----
            wz = const_pool.tile([128, 144], bf16)
            nc.gpsimd.memset(wz[:], 0.0)
            pw = psums_pool.tile([BATCH, 16], f32, tag="psum_s")
            for _ in range(8):
                nc.tensor.matmul(pw[:], wz[:, :BATCH], wz[:, 128:144], start=True, stop=True)

            # ---- resident constants ----
            w_sb = const_pool.tile([128, K_CHUNKS, EMBED_DIM], bf16)
            nc.sync.dma_start(out=w_sb[:], in_=w_t[:])
            a_sb = const_pool.tile([128, K_CHUNKS, SHARD], bf16)
            for g0, gsz in zip(GOFFS, GSIZES):
                nc.sync.dma_start(
                    out=a_sb[:, :, g0 : g0 + gsz],
                    in_=a_t[:, :, g0 : g0 + gsz],
                )
            qt_sb = const_pool.tile([128, 2, BATCH], f32)
            nc.sync.dma_start(out=qt_sb[:], in_=qt[:])
            qtn_sb = const_pool.tile([128, 2, BATCH], f32)
            nc.sync.dma_start(out=qtn_sb[:], in_=qtn[:])
            # one-hot staircase: column 31 of the window is all-1.0
            stair = const_pool.tile([128, 2 * BATCH - 1], bf16)
            nc.gpsimd.memset(stair[:], 0.0)
            nc.gpsimd.memset(stair[:, BATCH - 1 : BATCH], 2.0)
            stair8 = const_pool.tile([128, 2, 2 * BATCH], fp8)  # i-stride 64B (16B-aligned)
            nc.gpsimd.memset(stair8[:], 0.0)
            nc.gpsimd.memset(stair8[:, :, BATCH - 1 : BATCH], 2.0)
            # keep the PE p-state warm through the input-DMA window: a paced
            # Pool->PE trickle of tiny matmuls (bufs=2 chain sets the pace)
            for _ in range(18):
                tr = trk_pool.tile([128, 16], bf16, tag="tr")
                nc.gpsimd.memset(tr[:], 0.0)
                nc.tensor.matmul(pw[:], wz[:, :BATCH], tr[:], start=True, stop=True)

            copy_i = [0]
            p_tiles = {}

            def chunks(gsz):
                csz = [512] * (gsz // 512) + ([gsz % 512] if gsz % 512 else [])
                coff = [sum(csz[:i]) for i in range(len(csz))]
                return csz, coff

            def emit_proj(g):
                g0 = GOFFS[g]
                gsz = GSIZES[g]
                csz, coff = chunks(gsz)
                pt_ps = []
                for h in range(2):
                    ptp = psumt_pool.tile([128, gsz], f32, tag="ptp")
                    for c in range(len(csz)):
                        sl = slice(coff[c], coff[c] + csz[c])
                        gl = slice(g0 + coff[c], g0 + coff[c] + csz[c])
                        gl = slice(g0 + coff[c], g0 + coff[c] + csz[c])
                        for k in range(K_CHUNKS):
                            nc.tensor.matmul(
                                ptp[:, sl],
                                w_sb[:, k, 128 * h : 128 * (h + 1)],
                                a_sb[:, k, gl],
                                start=(k == 0), stop=(k == K_CHUNKS - 1),
                            )
                    pt_ps.append(ptp)
                p_tiles[g] = pt_ps

            def emit_copies(g):
                gsz = GSIZES[g]
                p_sb = p_pool.tile([128, 2, gsz], bf16, tag="p")
                for h in range(2):
                    eng = COPY_ENGINES[copy_i[0] % len(COPY_ENGINES)]
                    copy_i[0] += 1
                    if eng == 'v':
                        nc.vector.tensor_copy(out=p_sb[:, h, :], in_=p_tiles[g][h][:])
                    else:
                        nc.scalar.copy(p_sb[:, h, :], p_tiles[g][h][:])
                p_tiles[g] = p_sb

            def emit_produce_reduce(g):
                g0 = GOFFS[g]
                gsz = GSIZES[g]
                csz, coff = chunks(gsz)
                p_sb = p_tiles[g]
                psum_s = psums_pool.tile([BATCH, gsz], f32, tag="psum_s")

                # produce: fp8 (ACT/Pool streams) first, then DVE halves+folds
                t8s = {}
                for b in R_FP8:
                    t8 = abs8_pool.tile([128, 2, gsz], fp8, tag="abs8")
                    for h in range(2):
                        if b in R_ACT8 or h == 0:
                            nc.scalar.activation(
                                t8[:, h, :], p_sb[:, h, :], AF.Relu,
                                bias=qtn_sb[:, h, b : b + 1], scale=1.0,
                            )
                        else:
                            nc.gpsimd.tensor_scalar(
                                out=t8[:, h, :], in0=p_sb[:, h, :],
                                scalar1=qt_sb[:, h, b : b + 1], scalar2=0.0,
                                op0=AL.subtract, op1=AL.max,
                            )
                    t8s[b] = t8
                reds = {}
                for b in R_DVE_F + R_DVE_U:
                    halves = []
                    for h in range(2):
                        ab = absd_pool.tile([128, gsz], bf16, tag="absd")
                        nc.vector.tensor_scalar(
                            out=ab[:], in0=p_sb[:, h, :],
                            scalar1=qt_sb[:, h, b : b + 1], scalar2=0.0,
                            op0=AL.subtract, op1=AL.max,
                        )
                        halves.append(ab)
                    if b in R_DVE_F:
                        fd = absd_pool.tile([128, gsz], bf16, tag="fold")
                        nc.vector.tensor_tensor(
                            out=fd[:], in0=halves[0][:], in1=halves[1][:], op=AL.add
                        )
                        reds[b] = [fd]
                    else:
                        reds[b] = halves

                # reduces in producer-readiness order: DVE folded, unfolded, fp8
                order = [(b, False) for b in R_DVE_F + R_DVE_U] + [(b, True) for b in R_FP8]
                n_items = sum(len(reds[b]) for b, f8 in order if not f8) + len(R_FP8)
                it = 0
                first_mm = True
                for b, is8 in order:
                    if is8:
                        it += 1
                        last = it == n_items
                        for c in range(len(csz)):
                            sl = slice(coff[c], coff[c] + csz[c])
                            nc.tensor.matmul(
                                psum_s[:, sl],
                                stair8[:, :, BATCH - 1 - b : 2 * BATCH - 1 - b],
                                t8s[b][:, :, sl],
                                start=first_mm, stop=last,
                                perf_mode=DR, skip_group_check=True,
                            )
                        first_mm = False
                    else:
                        for r in reds[b]:
                            it += 1
                            last = it == n_items
                            for c in range(len(csz)):
                                sl = slice(coff[c], coff[c] + csz[c])
                                nc.tensor.matmul(
                                    psum_s[:, sl],
                                    stair[:, BATCH - 1 - b : 2 * BATCH - 1 - b],
                                    r[:, sl],
                                    start=first_mm, stop=last,
                                    skip_group_check=True,
                                )
                            first_mm = False

                # scores: PSUM -> SBUF f32 -> DRAM
                sr = sr_pool.tile([BATCH, gsz], f32, tag="sr")
                seng = SCOPY_ENGINES[g % len(SCOPY_ENGINES)]
                if seng == 'v':
                    nc.vector.tensor_copy(out=sr[:], in_=psum_s[:])
                else:
                    nc.scalar.copy(sr[:], psum_s[:])
                nc.sync.dma_start(out=st_out[:, g0 : g0 + gsz], in_=sr[:])

            # software-pipelined emission: proj runs one group ahead
            emit_proj(0)
            emit_copies(0)
            for g in range(1, N_GROUPS):
                emit_proj(g)
                emit_produce_reduce(g - 1)
                emit_copies(g)
            emit_produce_reduce(N_GROUPS - 1)

    nc.compile()
    return nc


def _get_program():
    if "nc" not in _CACHE:
        _CACHE["nc"] = _build_program()
    return _CACHE["nc"]


def _host_query_sum(ent_pkl, other_emb, proj_W, batch_input_ids, mp):
    """Exact replica of the reference's query path, on host (64 rows only)."""
    ids = np.concatenate([batch_input_ids[:, :mp], batch_input_ids[:, mp + 1 : 3]], axis=1)
    ids = ids.astype(np.int64)  # [B, 2]
    q = np.empty((BATCH, 2, EMBED_DIM), dtype=np.float32)
    for b in range(BATCH):
        for j in range(2):
            idx = int(ids[b, j])
            if idx == 0:
                row = other_emb[0]
            elif idx <= NUM_ENT:
                row = ent_pkl[idx - 1].astype(np.float32) @ proj_W.T.astype(np.float32)
            else:
                row = other_emb[idx - NUM_ENT]
            q[b, j] = row
    norm = np.sqrt((q * q).sum(-1, keepdims=True))
    q = q / np.maximum(norm, EPS)
    return q.sum(axis=1)  # [B, 256] float32


def kernel(ent_pkl, other_emb, proj_W, batch_input_ids, batch_mask_position, _timing=None):
    from concourse.bass_utils import run_bass_kernel_spmd

    ent_pkl = np.asarray(ent_pkl, dtype=np.float32)
    other_emb = np.asarray(other_emb, dtype=np.float32)
    proj_W = np.asarray(proj_W, dtype=np.float32)
    batch_input_ids = np.asarray(batch_input_ids)
    mp = int(np.asarray(batch_mask_position))

    q_sum = _host_query_sum(ent_pkl, other_emb, proj_W, batch_input_ids, mp)

    # score column 0: entity row = other_emb[0]
    col0 = -np.abs(q_sum - other_emb[0][None, :]).sum(-1)  # [B]

    # qt[d, h, b] = q_sum[b, 128h + d]
    qth = np.transpose(q_sum.T.reshape(2, 128, BATCH), (1, 0, 2))  # [128, 2, 32]
    qt_np = np.ascontiguousarray(qth.astype(np.float32))
    qtn_np = np.ascontiguousarray((-qth).astype(np.float32))

    w_np = np.ascontiguousarray(
        np.transpose(proj_W.reshape(EMBED_DIM, K_CHUNKS, 128), (2, 1, 0))
    ).astype(BF16)  # [128, 6, 256]

    in_maps = []
    for c in range(N_CORES):
        shard = ent_pkl[c * SHARD : (c + 1) * SHARD]  # [5000, 768]
        a_np = np.ascontiguousarray(np.transpose(
            shard.reshape(SHARD, K_CHUNKS, 128), (2, 1, 0)
        ).astype(BF16))  # [128, 6, SHARD]
        in_maps.append({"a_t": a_np, "w_t": w_np, "qt": qt_np, "qtn": qtn_np})

    nc = _get_program()
    kwargs = dict(_timing) if _timing else {}
    res = run_bass_kernel_spmd(nc, in_maps, list(range(N_CORES)), **kwargs)
    if _timing is not None:
        _CACHE["last_results"] = res

    # host correction: score = 2*sum(relu) - (colsum_P[e] - qsum[b])
    w_bf = proj_W.astype(BF16).astype(np.float32)  # [256, 768]
    w1 = w_bf.sum(axis=0)  # [768]
    qsum = q_sum.sum(-1).astype(np.float32)  # [B]
    s_ent = np.empty((BATCH, NUM_ENT), dtype=np.float32)
    for c in range(N_CORES):
        shard_bf = ent_pkl[c * SHARD : (c + 1) * SHARD].astype(BF16).astype(np.float32)
        colsum = shard_bf @ w1  # [SHARD]
        s_ent[:, c * SHARD : (c + 1) * SHARD] = (
            res.results[c]["st_out"][:, :SHARD] - colsum[None, :] + qsum[:, None]
        )
    out = np.empty((BATCH, NUM_ENT + 1), dtype=np.float32)
    out[:, 0] = col0
    out[:, 1:] = -s_ent
    return out
